# revision 1
# baseline (speedup 1.0000x reference)
"""Trainium2 Bass kernel for a dense transformer block (nn_Block_47888885351104).

Full inputs -> full outputs. Internally: data-parallel shard of batch B=256
across 8 NeuronCores (32 batches per core), one SPMD Bass/Tile program.

Per-core pipeline, per batch b (T=256 tokens = 2 partition tiles):
  LN1 (bn_stats + fused scale/bias ACT) -> transpose via PE matmul-with-identity
  -> QKV projections (heads packed 2-per-128-partition group)
  -> causal attention: scores^T[u,t] per head (row-packed pairs on the PE
     array), exp via ACT (1/sqrt(C) folded into q), triangular mask multiply,
     AV matmul with a ones-column appended to V so the softmax denominator
     falls out of the same matmul, normalize via stride-0 broadcast reciprocal
  -> transpose attn out, Wo projection (+bo via rank-1 matmul), residual
  -> LN2 -> MLP (W1, relu+b1 via ACT bias, W2, +b2 rank-1), residual -> out.
"""

import sys

sys.path.insert(0, "/opt/trn_rl_repo")

import numpy as np

import concourse.bass as bass
import concourse.mybir as mybir
import concourse.tile as tile
from concourse import bacc
from concourse.bass import ts
from concourse.bass_utils import run_bass_kernel_spmd
from concourse.masks import make_identity, make_upper_triangular

F32 = mybir.dt.float32
BF16 = mybir.dt.bfloat16
F32R = mybir.dt.float32r

B, T, C, H, HS, FF = 256, 256, 384, 6, 64, 1536
NCORES = 8
import os as _os
NB = int(_os.environ.get("KERNEL_NB", B // NCORES))  # batches per core
NT = T // 128             # 2 token tiles per batch
NC = C // 128             # 3 channel chunks
NG = H // 2               # 3 head groups (2 heads of 64 share a 128 tile)
NF = FF // 128            # 12 hidden chunks
EPS = 1e-5
ISCALE = float(1.0 / np.sqrt(np.float32(C)))

MM_DT = BF16              # matmul dtype for projection path (BF16 or F32R)

Act = mybir.ActivationFunctionType
Alu = mybir.AluOpType


def bcast(ap, n):
    """Broadcast the last (size-1) free dim of `ap` to n via a stride-0 AP."""
    new = [list(d) for d in ap.ap]
    assert new[-1][1] == 1
    new[-1] = [0, n]
    return bass.AP(ap.tensor, ap.offset, new)


def _emit(nc, tc, d):
    from contextlib import ExitStack

    ctx = ExitStack()
    const = ctx.enter_context(tc.tile_pool(name="const", bufs=1))
    wpool = ctx.enter_context(tc.tile_pool(name="weights", bufs=1))
    _g = lambda k, dflt: int(_os.environ.get(k, dflt))
    xp = ctx.enter_context(tc.tile_pool(name="xp", bufs=_g("XP_BUFS", 3)))
    sb = ctx.enter_context(tc.tile_pool(name="sb", bufs=_g("SB_BUFS", 3)))
    sb3 = ctx.enter_context(tc.tile_pool(name="sb3", bufs=_g("SB3_BUFS", 2)))
    # PSUM: one shared pool, every tile <= 2 banks (1024 f32), tag-shared.
    pp = ctx.enter_context(tc.tile_pool(name="pp", bufs=_g("PP_BUFS", 3), space="PSUM"))
    pa = ctx.enter_context(tc.tile_pool(name="pa", bufs=_g("PA_BUFS", 2), space="PSUM"))

    # ---- constants -------------------------------------------------------
    ident = const.tile([128, 128], MM_DT)
    make_identity(nc, ident[:])
    mask = const.tile([128, 128], BF16)  # mask[u, t] = 1 if t >= u else 0
    make_upper_triangular(nc, mask[:], val=1.0, diag=True)
    epst = const.tile([128, 1], F32)
    nc.vector.memset(epst[:], EPS)

    # ---- weights ---------------------------------------------------------
    def load_cast(dram_ap, shape, tag, dt=MM_DT):
        wf = wpool.tile(shape, F32, tag=tag + "_f")
        nc.sync.dma_start(wf[:], dram_ap)
        wb = wpool.tile(shape, dt, tag=tag)
        nc.vector.tensor_copy(wb[:], wf[:])
        return wb

    def load_qkv(dram_ap, tag):
        wf = wpool.tile([128, NC, H * HS], F32, tag=tag + "_f")
        dram_v = dram_ap.rearrange("h (cc p) s -> p cc h s", p=128)
        for cc in range(NC):
            nc.sync.dma_start(
                wf[:, cc].rearrange("p (h s) -> p h s", h=H), dram_v[:, cc])
        wb = wpool.tile([128, NC, H * HS], MM_DT, tag=tag)
        nc.vector.tensor_copy(wb[:], wf[:])
        return wb

    wk = load_qkv(d["Wk"], "wk")
    wq = load_qkv(d["Wq"], "wq")
    wv = load_qkv(d["Wv"], "wv")
    wo = load_cast(d["Wo"].rearrange("(cc p) c2 -> p cc c2", p=128),
                   [128, NC, C], "wo")
    w1 = load_cast(d["W1"].rearrange("(cc p) f -> p cc f", p=128),
                   [128, NC, FF], "w1")
    w2 = load_cast(d["W2"].rearrange("(fc p) c2 -> p fc c2", p=128),
                   [128, NF, C], "w2")
    bo_t = load_cast(d["bo"][None, :], [1, C], "bo")
    b2_t = load_cast(d["b2"][None, :], [1, C], "b2")
    b1_t = wpool.tile([128, NF], F32, tag="b1")
    nc.sync.dma_start(b1_t[:], d["b1"].rearrange("(fc p) -> p fc", p=128))
    ones1 = wpool.tile([1, 128], MM_DT, tag="ones1")
    nc.vector.memset(ones1[:], 1.0)

    x_d, out_d = d["x"], d["out"]

    # ---- LN helper -------------------------------------------------------
    def layernorm_T(xin, tag, evac_act):
        """xin: [128, NT, C] f32 -> hT [128, NC, T] MM_DT (normalized, T-major)."""
        stats = sb.tile([128, NT, 6], F32, tag=tag + "st")
        mv = sb.tile([128, NT, 2], F32, tag=tag + "mv")
        for tt in range(NT):
            nc.vector.bn_stats(stats[:, tt], xin[:, tt])
            nc.vector.bn_aggr(mv[:, tt], stats[:, tt])
        sd = sb.tile([128, NT], F32, tag=tag + "sd")
        nc.scalar.activation(sd[:], mv[:, :, 1], Act.Sqrt, bias=epst[:],
                             scale=float(C / (C - 1)))
        r = sb.tile([128, NT], F32, tag=tag + "r")
        nc.vector.reciprocal(r[:], sd[:])
        nmur = sb.tile([128, NT], F32, tag=tag + "nmur")
        nc.vector.tensor_tensor(out=nmur[:], in0=mv[:, :, 0], in1=r[:],
                                op=Alu.mult)
        nc.vector.tensor_scalar_mul(nmur[:], nmur[:], -1.0)

        hc = sb.tile([128, NT, C], MM_DT, tag=tag + "hc")
        for tt in range(NT):
            nc.scalar.activation(hc[:, tt], xin[:, tt], Act.Identity,
                                 bias=nmur[:, tt:tt + 1],
                                 scale=r[:, tt:tt + 1])
        ps = pp.tile([128, NC, NT, 128], F32, tag="big")
        for cc in range(NC):
            for tt in range(NT):
                nc.tensor.matmul(ps[:, cc, tt], hc[:, tt, ts(cc, 128)],
                                 ident[:], start=True, stop=True)
        hT = sb.tile([128, NC, T], MM_DT, tag=tag + "hT")
        if evac_act:
            nc.scalar.activation(hT[:], ps[:], Act.Copy)
        else:
            nc.vector.tensor_copy(hT[:], ps[:])
        return hT

    STAGE = int(_os.environ.get("KERNEL_STAGE", "9"))

    def emit_partial(src_ap, b):
        n = 1
        for _, cnt in src_ap.ap[1:]:
            n *= cnt
        pad = xp.tile([128, NT, C], F32, tag="outt")
        nc.vector.memset(pad[:], 0.0)
        nc.vector.tensor_copy(pad[:, 0, 0:n], src_ap)
        nc.sync.dma_start(out_d[b].rearrange("(tt p) c -> p tt c", p=128),
                          pad[:])

    # ---- per-batch pipeline ---------------------------------------------
    for b in range(NB):
        xb = xp.tile([128, NT, C], F32, tag="xb")
        nc.sync.dma_start(xb[:], x_d[b].rearrange("(tt p) c -> p tt c", p=128))

        hT = layernorm_T(xb, "ln1", False)
        if STAGE == 1:
            emit_partial(hT[:, 0, 0:128], b)
            continue

        # --- QKV projections ---
        k_ps = pp.tile([128, NG, T], F32, tag="big")
        q_ps = pp.tile([128, NG, T], F32, tag="big")
        for g in range(NG):
            for cc in range(NC):
                nc.tensor.matmul(k_ps[:, g], wk[:, cc, ts(g, 128)], hT[:, cc],
                                 start=(cc == 0), stop=(cc == NC - 1))
        for g in range(NG):
            for cc in range(NC):
                nc.tensor.matmul(q_ps[:, g], wq[:, cc, ts(g, 128)], hT[:, cc],
                                 start=(cc == 0), stop=(cc == NC - 1))
        kT = sb.tile([128, NG, T], MM_DT, tag="kT")
        nc.vector.tensor_copy(kT[:], k_ps[:])
        qT = sb.tile([128, NG, T], MM_DT, tag="qT")
        nc.scalar.activation(qT[:], q_ps[:], Act.Copy, scale=ISCALE)

        if STAGE == 25:
            emit_partial(qT[:, 0, 0:128], b)
            continue
        # v in [u, h*s] layout (+ ones column per head interleaved)
        v_ps = pp.tile([128, NT, 512], F32, tag="big")
        for uc in range(NT):
            for cc in range(NC):
                nc.tensor.matmul(v_ps[:, uc, 0:C], hT[:, cc, ts(uc, 128)],
                                 wv[:, cc], start=(cc == 0),
                                 stop=(cc == NC - 1))
        vb = sb.tile([128, NT, H, HS + 1], BF16, tag="vb")
        nc.vector.tensor_copy(
            vb[:, :, :, 0:HS],
            v_ps[:, :, 0:C].rearrange("p uc (h s) -> p uc h s", h=H))
        nc.vector.memset(vb[:, :, :, HS:HS + 1], 1.0)
        if STAGE == 2:
            emit_partial(vb[:, 0, 0, :], b)
            continue

        # --- scores: weiT[u, t] per head; row-packed head pairs ---
        wei0 = sb3.tile([128, H, 256], BF16, tag="wei0")  # u0, t 0:256
        wei1 = sb3.tile([128, H, 128], BF16, tag="wei1")  # u1, t 128:256
        for g in range(NG):
            # concurrent row-group matmuls (base partition 0 / 64) must land
            # in different PSUM banks: sub dim strided by one bank (512 f32).
            s_ps = pp.tile([128, 2, 512], F32, tag="big")
            for sub in range(2):
                nc.tensor.matmul(s_ps[:, sub, 0:256],
                                 qT[ts(sub, 64), g, 0:128],
                                 kT[ts(sub, 64), g, :], start=True, stop=True)
                nc.tensor.matmul(s_ps[:, sub, 256:384],
                                 qT[ts(sub, 64), g, 128:256],
                                 kT[ts(sub, 64), g, 128:256], start=True,
                                 stop=True)
            nc.scalar.activation(wei0[:, ts(g, 2)], s_ps[:, :, 0:256],
                                 Act.Exp)
            nc.scalar.activation(wei1[:, ts(g, 2)], s_ps[:, :, 256:384],
                                 Act.Exp)
        if STAGE == 35:
            emit_partial(wei0[:, 0, 0:128], b)
            continue
        mb0 = mask[:, None, :].to_broadcast((128, H, 128))
        nc.vector.tensor_tensor(out=wei0[:, :, 0:128], in0=wei0[:, :, 0:128],
                                in1=mb0, op=Alu.mult)
        nc.vector.tensor_tensor(out=wei1[:], in0=wei1[:], in1=mb0,
                                op=Alu.mult)
        if STAGE == 3:
            emit_partial(wei0[:, 0, 0:128], b)
            continue

        # --- AV with denominator column ---
        o_ps = pp.tile([128, NT, 512], F32, tag="big")
        o_v = o_ps[:, :, 0:H * (HS + 1)].rearrange(
            "p tc (h z) -> p tc h z", z=HS + 1)
        for h in range(H):
            nc.tensor.matmul(o_v[:, 0, h], wei0[:, h, 0:128], vb[:, 0, h],
                             start=True, stop=True)
            nc.tensor.matmul(o_v[:, 1, h], wei0[:, h, 128:256], vb[:, 0, h],
                             start=True, stop=False)
            nc.tensor.matmul(o_v[:, 1, h], wei1[:, h], vb[:, 1, h],
                             start=False, stop=True)

        rec = sb.tile([128, NT, H, 1], F32, tag="rec")
        nc.vector.reciprocal(rec[:], o_v[:, :, :, HS:HS + 1])
        att = sb.tile([128, NT, H, HS], MM_DT, tag="att")
        nc.vector.tensor_tensor(out=att[:], in0=o_v[:, :, :, 0:HS],
                                in1=bcast(rec[:], HS), op=Alu.mult)
        if STAGE == 4:
            emit_partial(att[:, 0, 0, :], b)
            continue

        # --- transpose attn out -> [hs, t] ---
        at_ps = pp.tile([128, NC, NT, 128], F32, tag="big")
        attf = att[:].rearrange("p tt h s -> p tt (h s)")
        for cc in range(NC):
            for tt in range(NT):
                nc.tensor.matmul(at_ps[:, cc, tt], attf[:, tt, ts(cc, 128)],
                                 ident[:], start=True, stop=True)
        attT = sb.tile([128, NC, T], MM_DT, tag="attT")
        nc.scalar.activation(attT[:], at_ps[:], Act.Copy)

        # --- Wo projection + bo + residual ---
        pr_ps = pp.tile([128, NT, 512], F32, tag="big")
        for tt in range(NT):
            for cc in range(NC):
                nc.tensor.matmul(pr_ps[:, tt, 0:C], attT[:, cc, ts(tt, 128)],
                                 wo[:, cc], start=(cc == 0), stop=False)
            nc.tensor.matmul(pr_ps[:, tt, 0:C], ones1[0:1, :],
                             bo_t[:], start=False, stop=True)
        y = xp.tile([128, NT, C], F32, tag="y")
        nc.vector.tensor_tensor(out=y[:], in0=pr_ps[:, :, 0:C], in1=xb[:],
                                op=Alu.add)

        if STAGE == 5:
            emit_partial(y[:, 0, 0:128], b)
            continue
        hT2 = layernorm_T(y, "ln2", True)
        if STAGE == 6:
            emit_partial(hT2[:, 0, 0:128], b)
            continue

        # --- MLP: a1T[f, t] = relu(W1.T @ h2T + b1) ---
        # 4 f-chunks share one 2-bank psum tile; relu+bias evac per chunk
        # (bias differs per chunk), but matmuls stream without pool churn.
        a1 = sb3.tile([128, NF, T], MM_DT, tag="a1")
        for fq in range(NF // 2):
            a_ps = pa.tile([128, 2, T], F32, tag="aps")
            for j in range(2):
                fc = fq * 2 + j
                for cc in range(NC):
                    nc.tensor.matmul(a_ps[:, j], w1[:, cc, ts(fc, 128)],
                                     hT2[:, cc], start=(cc == 0),
                                     stop=(cc == NC - 1))
            for j in range(2):
                fc = fq * 2 + j
                nc.scalar.activation(a1[:, fc], a_ps[:, j], Act.Relu,
                                     bias=b1_t[:, fc:fc + 1], scale=1.0)
        if STAGE == 7:
            emit_partial(a1[:, 0, 0:128], b)
            continue

        # --- W2 + b2 + residual -> out ---
        ff_ps = pp.tile([128, NT, 512], F32, tag="big")
        for tt in range(NT):
            for fc in range(NF):
                nc.tensor.matmul(ff_ps[:, tt, 0:C], a1[:, fc, ts(tt, 128)],
                                 w2[:, fc], start=(fc == 0), stop=False)
            nc.tensor.matmul(ff_ps[:, tt, 0:C], ones1[0:1, :],
                             b2_t[:], start=False, stop=True)
        out_t = xp.tile([128, NT, C], F32, tag="outt")
        nc.vector.tensor_tensor(out=out_t[:], in0=ff_ps[:, :, 0:C], in1=y[:],
                                op=Alu.add)
        nc.sync.dma_start(out_d[b].rearrange("(tt p) c -> p tt c", p=128),
                          out_t[:])

    ctx.close()


def _build_program():
    nc = bacc.Bacc("TRN2", target_bir_lowering=False, debug=False)
    d = {}
    specs = {
        "x": [NB, T, C], "Wk": [H, C, HS], "Wq": [H, C, HS],
        "Wv": [H, C, HS], "Wo": [C, C], "bo": [C], "W1": [C, FF],
        "b1": [FF], "W2": [FF, C], "b2": [C],
    }
    for name, shape in specs.items():
        d[name] = nc.dram_tensor(name, shape, F32, kind="ExternalInput").ap()
    d["out"] = nc.dram_tensor("out", [NB, T, C], F32,
                              kind="ExternalOutput").ap()
    with tile.TileContext(nc) as tc:
        _emit(nc, tc, d)
    nc.compile()
    return nc


_CACHED_NC = None


def _get_program():
    global _CACHED_NC
    if _CACHED_NC is None:
        _CACHED_NC = _build_program()
    return _CACHED_NC


def kernel(**inputs):
    nc = _get_program()
    x = np.ascontiguousarray(np.asarray(inputs["x"], dtype=np.float32))
    shared = {k: np.ascontiguousarray(np.asarray(inputs[k], np.float32))
              for k in ("Wk", "Wq", "Wv", "Wo", "bo", "W1", "b1", "W2", "b2")}
    in_maps = [dict(shared, x=x[c * NB:(c + 1) * NB]) for c in range(NCORES)]
    res = run_bass_kernel_spmd(nc, in_maps, core_ids=list(range(NCORES)))
    out = np.concatenate([res.results[c]["out"] for c in range(NCORES)],
                         axis=0)
    return out.astype(np.float32)

